# revision 9
# baseline (speedup 1.0000x reference)
"""DGN temporal GNN conv kernel for Trainium2 (8 NeuronCores).

Math (per timestep t):
    w_e(t) = edge_weight[e] if edge_time[e] <= node_time[t] else 0
    agg[n] = sum_{e: dst[e]==n} w_e(t) * x[t, src[e]]
    out[t] = agg @ W + b

Device strategy:
  - Destination nodes sharded across 8 cores (6250 each); every core runs all
    8 timesteps -> statistically identical load, so one compile-time schedule
    (chunk counts maxed over cores per 64-node group) serves all cores (one
    NEFF, SPMD).
  - x rows gathered by GPSIMD dma_gather from per-timestep bf16 tables in HBM
    (row duplicated to 256 bytes; int16 indices, src split at 32768 into
    lo/hi gather calls).
  - Scatter-sum on PE: per 128-edge chunk a [128e x 64slot] weighted one-hot
    (one DVE tensor_scalar: (iota==key)*w) is the moving matmul operand, the
    gathered rows the stationary one.  All chunks of a 64-node group (lo and
    hi interleaved) accumulate one PSUM region, drained once by ACT into a
    feature-major SBUF aggT tile.
  - Tail per timestep: aggT @ W (W stationary), +bias (ACT), PE-transpose to
    row-major, DMA out.
"""

import os
import numpy as np

T, N, E, D = 8, 50000, 800000, 64
NC = 8
RANGE = N // NC          # 6250 dst nodes per core
GR = 64                  # nodes per group (PSUM region [64 feat, 64 slot])
NGRP = (RANGE + GR - 1) // GR   # 98 groups per timestep
SLOTS_PER_T = NGRP * GR  # 6272 aggT slots per timestep
SPLIT = 32768            # src split for int16 gather indices
CHUNK = 128              # edges per chunk (PE contraction dim)
SB_CHUNKS = 96           # max chunks per super-batch (msg SBUF tile size)
PAD_KEY = 99.0


# ---------------------------------------------------------------------------
# Host-side schedule
# ---------------------------------------------------------------------------

def _build_schedule(edge_index, edge_time, node_time, edge_weight):
    src = np.asarray(edge_index[0], dtype=np.int64)
    dst = np.asarray(edge_index[1], dtype=np.int64)
    edge_time = np.asarray(edge_time, dtype=np.float32)
    edge_weight = np.asarray(edge_weight, dtype=np.float32)
    node_time = np.asarray(node_time, dtype=np.float32)
    core_of = dst // RANGE

    per = {}
    counts = np.zeros((NC, T, 2, NGRP), dtype=np.int64)
    for k in range(NC):
        m_core = core_of == k
        s_k = src[m_core]
        d_k = dst[m_core] - k * RANGE
        et_k = edge_time[m_core]
        w_k = edge_weight[m_core]
        g_k = d_k // GR
        slot_k = d_k % GR
        half_k = (s_k >= SPLIT).astype(np.int64)
        idx_k = np.where(half_k == 1, s_k - SPLIT, s_k)
        for t in range(T):
            act = et_k <= node_time[t]
            for h in (0, 1):
                m = act & (half_k == h)
                order = np.argsort(g_k[m], kind="stable")
                per[(k, t, h)] = (g_k[m][order], slot_k[m][order],
                                  idx_k[m][order], w_k[m][order])
                counts[k, t, h] = np.bincount(g_k[m], minlength=NGRP)

    nch = -(-counts // CHUNK)
    nch = nch.max(axis=0)                      # [T, 2, NGRP]
    nch[:, 0, :] = np.maximum(nch[:, 0, :], 1)  # lo >= 1 so drain inits aggT

    # Super-batches: consecutive groups of one t, total chunks <= SB_CHUNKS.
    # Stream (idx/key/w) order: per sb, all lo chunks (group order), then all
    # hi chunks.  Processing order: per group, its lo then hi chunks.
    sbs = []          # list of dicts
    chunk_base = np.zeros((T, 2, NGRP), dtype=np.int64)  # stream chunk id
    n_chunks = 0
    for t in range(T):
        g = 0
        while g < NGRP:
            g1 = g
            tot = 0
            while g1 < NGRP:
                c = int(nch[t, 0, g1] + nch[t, 1, g1])
                if tot + c > SB_CHUNKS and g1 > g:
                    break
                tot += c
                g1 += 1
            groups = list(range(g, g1))
            lo0 = n_chunks
            for gg in groups:
                chunk_base[t, 0, gg] = n_chunks
                n_chunks += int(nch[t, 0, gg])
            hi0 = n_chunks
            for gg in groups:
                chunk_base[t, 1, gg] = n_chunks
                n_chunks += int(nch[t, 1, gg])
            sbs.append({"t": t, "groups": groups,
                        "lo": (lo0, hi0), "hi": (hi0, n_chunks)})
            g = g1
    n_slots = n_chunks * CHUNK

    idx_stream = np.zeros((NC, n_slots), dtype=np.int16)
    key_stream = np.full((NC, n_chunks, CHUNK), PAD_KEY, dtype=np.float32)
    w_stream = np.zeros((NC, n_chunks, CHUNK), dtype=np.float32)
    for k in range(NC):
        for t in range(T):
            for h in (0, 1):
                g_a, slot_a, idx_a, w_a = per[(k, t, h)]
                if len(g_a) == 0:
                    continue
                cg = counts[k, t, h]
                grp_off = np.concatenate([[0], np.cumsum(cg)[:-1]])
                r = np.arange(len(g_a)) - grp_off[g_a]
                ci = chunk_base[t, h, g_a] + r // CHUNK
                lane = r % CHUNK
                idx_stream[k, ci * CHUNK + lane] = idx_a.astype(np.int16)
                key_stream[k, ci, lane] = slot_a.astype(np.float32)
                w_stream[k, ci, lane] = w_a

    sched = {"sbs": sbs, "nch": nch, "chunk_base": chunk_base,
             "n_chunks": n_chunks, "n_slots": n_slots}
    return sched, (idx_stream, key_stream, w_stream)


def _pack_idx(idx_stream):
    """[NC, n_slots] -> [NC, 128, n_slots//16]: slot j at partition j%16,
    col j//16, replicated into all 8 groups of 16 partitions."""
    nc_, n_slots = idx_stream.shape
    cols = n_slots // 16
    wrapped = idx_stream.reshape(nc_, cols, 16).transpose(0, 2, 1)
    return np.ascontiguousarray(np.tile(wrapped, (1, 8, 1)))


# ---------------------------------------------------------------------------
# Numpy emulation of the device schedule (host-logic validation)
# ---------------------------------------------------------------------------

def emulate(x, edge_index, edge_time, node_time, edge_weight, W, b):
    sched, (idx_s, key_s, w_s) = _build_schedule(
        edge_index, edge_time, node_time, edge_weight)
    xf = np.asarray(x, dtype=np.float32)
    Wf = np.asarray(W, dtype=np.float32)
    bf = np.asarray(b, dtype=np.float32)
    nch = sched["nch"]
    chunk_base = sched["chunk_base"]
    out = np.zeros((T, N, D), dtype=np.float32)
    iota = np.arange(GR, dtype=np.float32)
    for k in range(NC):
        aggT = np.zeros((D, T * SLOTS_PER_T), dtype=np.float32)
        for sb in sched["sbs"]:
            t = sb["t"]
            for g in sb["groups"]:
                psum = np.zeros((D, GR), dtype=np.float32)
                for h in (0, 1):
                    for c in range(int(nch[t, h, g])):
                        ci = int(chunk_base[t, h, g]) + c
                        idx = idx_s[k, ci * CHUNK:(ci + 1) * CHUNK].astype(np.int64)
                        base = SPLIT if h else 0
                        msg = xf[t, base + idx, :]
                        key = key_s[k, ci]
                        w = w_s[k, ci]
                        sel = (key[:, None] == iota[None, :]) * w[:, None]
                        psum += msg.T @ sel
                sl = t * SLOTS_PER_T + g * GR
                aggT[:, sl:sl + GR] = psum
        for t in range(T):
            block = aggT[:, t * SLOTS_PER_T:(t + 1) * SLOTS_PER_T]
            outT = Wf.T @ block + bf[:, None]
            out[t, k * RANGE:(k + 1) * RANGE, :] = outT[:, :RANGE].T
    return out


# ---------------------------------------------------------------------------
# Bass kernel builder
# ---------------------------------------------------------------------------

def build_tile_kernel(tc, out_ap, ins, sched):
    """ins: dict with xt0..xt7 [N,128] bf16, idx [128, n_slots//16] i16,
    key/wgt [128, n_chunks] f32, iota [128, 64] bf16, wmat [64, 64] bf16,
    bias [64, 1] f32, ident [64, 64] f32.  out_ap: [T*RANGE, 64] f32."""
    from contextlib import ExitStack
    from concourse import bass, tile, mybir
    dt = mybir.dt
    nc = tc.nc
    nch = sched["nch"]
    chunk_base = sched["chunk_base"]

    with ExitStack() as ctx:
        const_p = ctx.enter_context(tc.tile_pool(name="const", bufs=1))
        msg_p = ctx.enter_context(tc.tile_pool(name="msg", bufs=2))
        aux_p = ctx.enter_context(tc.tile_pool(name="aux", bufs=2))
        sel_p = ctx.enter_context(tc.tile_pool(name="sel", bufs=4))
        agg_p = ctx.enter_context(tc.tile_pool(name="agg", bufs=1))
        stage_p = ctx.enter_context(tc.tile_pool(name="stage", bufs=3))
        psum_p = ctx.enter_context(tc.tile_pool(name="psum", bufs=4, space="PSUM"))
        psumt_p = ctx.enter_context(tc.tile_pool(name="psumt", bufs=2, space="PSUM"))

        iota_t = const_p.tile([128, GR], dt.bfloat16, tag="iota")
        nc.sync.dma_start(iota_t[:], ins["iota"][:])
        wmat_t = const_p.tile([D, D], dt.bfloat16, tag="wmat")
        nc.sync.dma_start(wmat_t[:], ins["wmat"][:])
        bias_t = const_p.tile([D, 1], dt.float32, tag="bias")
        nc.sync.dma_start(bias_t[:], ins["bias"][:])
        ident_t = const_p.tile([D, D], dt.float32, tag="ident")
        nc.sync.dma_start(ident_t[:], ins["ident"][:])

        aggT = agg_p.tile([D, T * SLOTS_PER_T], dt.bfloat16, tag="aggT")

        xt = [ins[f"xt{t}"] for t in range(T)]

        for sb in sched["sbs"]:
            t = sb["t"]
            lo0, lo1 = sb["lo"]
            hi0, hi1 = sb["hi"]
            nb = hi1 - lo0                     # total chunks in super-batch
            msg = msg_p.tile([128, SB_CHUNKS, 128], dt.bfloat16, tag="msg")
            # gather lo / hi
            for (c0, c1, base) in ((lo0, lo1, 0), (hi0, hi1, SPLIT)):
                nchk = c1 - c0
                if nchk == 0:
                    continue
                nidx = nchk * CHUNK
                idx_t = aux_p.tile([128, SB_CHUNKS * 8], dt.int16, tag="idx")
                nc.sync.dma_start(idx_t[:, :nidx // 16],
                                  ins["idx"][:, c0 * 8:c0 * 8 + nidx // 16])
                src_ap = xt[t][SPLIT:N, :] if base else xt[t][0:SPLIT, :]
                nc.gpsimd.dma_gather(
                    out_ap=msg[:, c0 - lo0:c0 - lo0 + nchk, :],
                    in_ap=src_ap,
                    idxs_ap=idx_t[:, :nidx // 16],
                    num_idxs=nidx,
                    num_idxs_reg=nidx,
                    elem_size=128,
                    single_packet=False,
                )
            key_t = aux_p.tile([128, SB_CHUNKS], dt.float32, tag="key")
            nc.sync.dma_start(key_t[:, :nb], ins["key"][:, lo0:lo0 + nb])
            w_t = aux_p.tile([128, SB_CHUNKS], dt.float32, tag="wgt")
            nc.sync.dma_start(w_t[:, :nb], ins["wgt"][:, lo0:lo0 + nb])

            for g in sb["groups"]:
                n_lo = int(nch[t, 0, g])
                n_hi = int(nch[t, 1, g])
                ntot = n_lo + n_hi
                psum = psum_p.tile([D, GR], dt.float32, tag="grp")
                done = 0
                for h, n_h in ((0, n_lo), (1, n_hi)):
                    cb = int(chunk_base[t, h, g])
                    for c in range(n_h):
                        ci = cb + c              # stream chunk id
                        pos = ci - lo0           # position in msg tile
                        sel = sel_p.tile([128, GR], dt.bfloat16, tag="sel")
                        nc.vector.tensor_scalar(
                            sel[:], iota_t[:],
                            key_t[:, ci - lo0:ci - lo0 + 1],
                            w_t[:, ci - lo0:ci - lo0 + 1],
                            mybir.AluOpType.is_equal, mybir.AluOpType.mult)
                        nc.tensor.matmul(
                            psum[:], msg[:, pos, 0:D], sel[:],
                            start=(done == 0), stop=(done == ntot - 1))
                        done += 1
                sl = t * SLOTS_PER_T + g * GR
                nc.scalar.activation(aggT[:, sl:sl + GR], psum[:],
                                     mybir.ActivationFunctionType.Copy)

        # Tail: per timestep @W, +bias, transpose, write out.
        for t in range(T):
            for s in range(0, SLOTS_PER_T, 512):
                w512 = min(512, SLOTS_PER_T - s)
                psw = psumt_p.tile([D, 512], dt.float32, tag="psw")
                nc.tensor.matmul(psw[:, :w512], wmat_t[:],
                                 aggT[:, t * SLOTS_PER_T + s:
                                      t * SLOTS_PER_T + s + w512],
                                 start=True, stop=True)
                outTs = stage_p.tile([D, 512], dt.float32, tag="outTs")
                nc.scalar.activation(outTs[:, :w512], psw[:, :w512],
                                     mybir.ActivationFunctionType.Identity,
                                     bias=bias_t[:])
                for s1 in range(0, w512, 128):
                    node0 = s + s1               # within this t's 6272 slots
                    if node0 >= RANGE:
                        break
                    nrow = min(128, RANGE - node0)
                    pst = psumt_p.tile([128, D], dt.float32, tag="pst")
                    nc.tensor.transpose(pst[:], outTs[:, s1:s1 + 128],
                                        ident_t[:])
                    st = stage_p.tile([128, D], dt.float32, tag="st")
                    nc.vector.tensor_copy(st[:], pst[:])
                    nc.sync.dma_start(
                        out_ap[t * RANGE + node0:t * RANGE + node0 + nrow, :],
                        st[:nrow, :])


# ---------------------------------------------------------------------------
# Top-level kernel
# ---------------------------------------------------------------------------

_CACHE = {}


def _declare_io(nc, dt, n_chunks, n_slots, null=False):
    in_aps = {}
    for t in range(T):
        in_aps[f"xt{t}"] = nc.dram_tensor(
            f"xt{t}", [N, 128], dt.bfloat16, kind="ExternalInput").ap()
    in_aps["idx"] = nc.dram_tensor(
        "idx", [128, n_slots // 16], dt.int16, kind="ExternalInput").ap()
    in_aps["key"] = nc.dram_tensor(
        "key", [128, n_chunks], dt.float32, kind="ExternalInput").ap()
    in_aps["wgt"] = nc.dram_tensor(
        "wgt", [128, n_chunks], dt.float32, kind="ExternalInput").ap()
    in_aps["iota"] = nc.dram_tensor(
        "iota", [128, GR], dt.bfloat16, kind="ExternalInput").ap()
    in_aps["wmat"] = nc.dram_tensor(
        "wmat", [D, D], dt.bfloat16, kind="ExternalInput").ap()
    in_aps["bias"] = nc.dram_tensor(
        "bias", [D, 1], dt.float32, kind="ExternalInput").ap()
    in_aps["ident"] = nc.dram_tensor(
        "ident", [D, D], dt.float32, kind="ExternalInput").ap()
    shape = [128, D] if null else [T * RANGE, D]
    out_ap = nc.dram_tensor("out", shape, dt.float32, kind="ExternalOutput").ap()
    return in_aps, out_ap


def _get_state(edge_index, edge_time, node_time, edge_weight):
    from concourse import bacc, tile, mybir
    dt = mybir.dt
    key = (edge_index.tobytes(), edge_time.tobytes(), node_time.tobytes(),
           edge_weight.tobytes())
    key = hash(key)
    if _CACHE.get("key") == key:
        return _CACHE["state"]

    sched, (idx_s, key_s, w_s) = _build_schedule(
        edge_index, edge_time, node_time, edge_weight)
    n_chunks, n_slots = sched["n_chunks"], sched["n_slots"]

    nc = bacc.Bacc("TRN2", target_bir_lowering=False, debug=False,
                   enable_asserts=False)
    in_aps, out_ap = _declare_io(nc, dt, n_chunks, n_slots)
    with tile.TileContext(nc) as tc:
        build_tile_kernel(tc, out_ap, in_aps, sched)
    if not nc.is_finalized():
        nc.finalize()

    # Null kernel: same inputs, trivial body (for transfer-overhead baseline).
    nc0 = bacc.Bacc("TRN2", target_bir_lowering=False, debug=False,
                    enable_asserts=False)
    in_aps0, out_ap0 = _declare_io(nc0, dt, n_chunks, n_slots, null=True)
    with tile.TileContext(nc0) as tc0:
        from contextlib import ExitStack
        with ExitStack() as c0:
            p0 = c0.enter_context(tc0.tile_pool(name="p0", bufs=1))
            t0_ = p0.tile([128, D], dt.float32, tag="t0")
            nc0.vector.memset(t0_[:], 0.0)
            nc0.sync.dma_start(t0_[0:D, :], in_aps0["ident"][:])
            nc0.sync.dma_start(out_ap0[:], t0_[:])
    if not nc0.is_finalized():
        nc0.finalize()

    state = {"sched": sched, "idx_s": idx_s, "key_s": key_s, "w_s": w_s,
             "nc": nc, "nc0": nc0,
             "idx_packed": _pack_idx(idx_s),
             "key_packed": key_s.transpose(0, 2, 1).copy(),
             "w_packed": w_s.transpose(0, 2, 1).copy()}
    _CACHE["key"] = key
    _CACHE["state"] = state
    return state


def _make_in_maps(state, x, W, b):
    import ml_dtypes
    bf16 = ml_dtypes.bfloat16
    xb = np.asarray(x).astype(bf16)
    xtab = np.concatenate([xb, xb], axis=2)               # [T, N, 128]
    iota_np = np.tile(np.arange(GR, dtype=np.float32)[None, :],
                      (128, 1)).astype(bf16)
    wmat_np = np.asarray(W).astype(bf16)
    bias_np = np.asarray(b).astype(np.float32).reshape(D, 1)
    ident_np = np.eye(D, dtype=np.float32)
    in_maps = []
    for k in range(NC):
        m = {f"xt{t}": xtab[t] for t in range(T)}
        m["idx"] = state["idx_packed"][k]
        m["key"] = state["key_packed"][k]
        m["wgt"] = state["w_packed"][k]
        m["iota"] = iota_np
        m["wmat"] = wmat_np
        m["bias"] = bias_np
        m["ident"] = ident_np
        in_maps.append(m)
    return in_maps


def kernel(x, edge_index, edge_time, node_time, edge_weight, W, b):
    from concourse.bass_utils import run_bass_kernel_spmd
    edge_index = np.asarray(edge_index)
    edge_time = np.asarray(edge_time)
    node_time = np.asarray(node_time)
    edge_weight = np.asarray(edge_weight)
    state = _get_state(edge_index, edge_time, node_time, edge_weight)
    in_maps = _make_in_maps(state, x, W, b)
    res = run_bass_kernel_spmd(state["nc"], in_maps, core_ids=list(range(NC)))
    out = np.zeros((T, N, D), dtype=np.float32)
    for k in range(NC):
        o = res.results[k]["out"].reshape(T, RANGE, D)
        out[:, k * RANGE:(k + 1) * RANGE, :] = o
    _CACHE["last_results"] = res
    return out


def null_run(x, edge_index, edge_time, node_time, edge_weight, W, b):
    """Same input transfer volume, trivial compute (timing baseline)."""
    from concourse.bass_utils import run_bass_kernel_spmd
    state = _get_state(np.asarray(edge_index), np.asarray(edge_time),
                       np.asarray(node_time), np.asarray(edge_weight))
    in_maps = _make_in_maps(state, x, W, b)
    res = run_bass_kernel_spmd(state["nc0"], in_maps, core_ids=list(range(NC)))
    return res.results[0]["out"]


# revision 13
# speedup vs baseline: 3211.3977x; 3211.3977x over previous
"""DGN temporal GNN conv kernel for Trainium2 (8 NeuronCores).

Math (per timestep t):
    w_e(t) = edge_weight[e] if edge_time[e] <= node_time[t] else 0
    agg[n] = sum_{e: dst[e]==n} w_e(t) * x[t, src[e]]
    out[t] = agg @ W + b

Device strategy:
  - Destination nodes sharded across 8 cores (6250 each); every core runs all
    8 timesteps -> statistically identical load, so one compile-time schedule
    (chunk counts maxed over cores per 64-node group) serves all cores (one
    NEFF, SPMD).
  - x rows gathered by GPSIMD dma_gather from per-timestep bf16 tables in HBM
    (row duplicated to 256 bytes; int16 indices, src split at 32768 into
    lo/hi gather calls).
  - Scatter-sum on PE: per 128-edge chunk a [128e x 64slot] weighted one-hot
    (one DVE tensor_scalar: (iota==key)*w) is the moving matmul operand, the
    gathered rows the stationary one.  All chunks of a 64-node group (lo and
    hi interleaved) accumulate one PSUM region, drained once by ACT into a
    feature-major SBUF aggT tile.
  - Tail per timestep: aggT @ W (W stationary), +bias (ACT), PE-transpose to
    row-major, DMA out.
"""

import os
import numpy as np

T, N, E, D = 8, 50000, 800000, 64
NC = 8
RANGE = N // NC          # 6250 dst nodes per core
GR = 64                  # nodes per group (PSUM region [64 feat, 64 slot])
NGRP = (RANGE + GR - 1) // GR   # 98 groups per timestep
SLOTS_PER_T = NGRP * GR  # 6272 aggT slots per timestep
SPLIT = 32768            # src split for int16 gather indices
CHUNK = 128              # edges per chunk (PE contraction dim)
SB_CHUNKS = 96           # max chunks per super-batch (msg SBUF tile size)
PAD_KEY = 99.0


# ---------------------------------------------------------------------------
# Host-side schedule
# ---------------------------------------------------------------------------

def _build_schedule(edge_index, edge_time, node_time, edge_weight):
    src = np.asarray(edge_index[0], dtype=np.int64)
    dst = np.asarray(edge_index[1], dtype=np.int64)
    edge_time = np.asarray(edge_time, dtype=np.float32)
    edge_weight = np.asarray(edge_weight, dtype=np.float32)
    node_time = np.asarray(node_time, dtype=np.float32)
    core_of = dst // RANGE

    per = {}
    counts = np.zeros((NC, T, 2, NGRP), dtype=np.int64)
    for k in range(NC):
        m_core = core_of == k
        s_k = src[m_core]
        d_k = dst[m_core] - k * RANGE
        et_k = edge_time[m_core]
        w_k = edge_weight[m_core]
        g_k = d_k // GR
        slot_k = d_k % GR
        half_k = (s_k >= SPLIT).astype(np.int64)
        idx_k = np.where(half_k == 1, s_k - SPLIT, s_k)
        for t in range(T):
            act = et_k <= node_time[t]
            for h in (0, 1):
                m = act & (half_k == h)
                order = np.argsort(g_k[m], kind="stable")
                per[(k, t, h)] = (g_k[m][order], slot_k[m][order],
                                  idx_k[m][order], w_k[m][order])
                counts[k, t, h] = np.bincount(g_k[m], minlength=NGRP)

    nch = -(-counts // CHUNK)
    nch = nch.max(axis=0)                      # [T, 2, NGRP]
    nch[:, 0, :] = np.maximum(nch[:, 0, :], 1)  # lo >= 1 so drain inits aggT

    # Super-batches: consecutive groups of one t, total chunks <= SB_CHUNKS.
    # Stream (idx/key/w) order: per sb, all lo chunks (group order), then all
    # hi chunks.  Processing order: per group, its lo then hi chunks.
    sbs = []          # list of dicts
    chunk_base = np.zeros((T, 2, NGRP), dtype=np.int64)  # stream chunk id
    n_chunks = 0
    for t in range(T):
        g = 0
        while g < NGRP:
            g1 = g
            tot = 0
            while g1 < NGRP:
                c = int(nch[t, 0, g1] + nch[t, 1, g1])
                if tot + c > SB_CHUNKS and g1 > g:
                    break
                tot += c
                g1 += 1
            groups = list(range(g, g1))
            lo0 = n_chunks
            for gg in groups:
                chunk_base[t, 0, gg] = n_chunks
                n_chunks += int(nch[t, 0, gg])
            hi0 = n_chunks
            for gg in groups:
                chunk_base[t, 1, gg] = n_chunks
                n_chunks += int(nch[t, 1, gg])
            sbs.append({"t": t, "groups": groups,
                        "lo": (lo0, hi0), "hi": (hi0, n_chunks)})
            g = g1
    n_slots = n_chunks * CHUNK

    idx_stream = np.zeros((NC, n_slots), dtype=np.int16)
    key_stream = np.full((NC, n_chunks, CHUNK), PAD_KEY, dtype=np.float32)
    w_stream = np.zeros((NC, n_chunks, CHUNK), dtype=np.float32)
    for k in range(NC):
        for t in range(T):
            for h in (0, 1):
                g_a, slot_a, idx_a, w_a = per[(k, t, h)]
                if len(g_a) == 0:
                    continue
                cg = counts[k, t, h]
                grp_off = np.concatenate([[0], np.cumsum(cg)[:-1]])
                r = np.arange(len(g_a)) - grp_off[g_a]
                ci = chunk_base[t, h, g_a] + r // CHUNK
                lane = r % CHUNK
                idx_stream[k, ci * CHUNK + lane] = idx_a.astype(np.int16)
                key_stream[k, ci, lane] = slot_a.astype(np.float32)
                w_stream[k, ci, lane] = w_a

    sched = {"sbs": sbs, "nch": nch, "chunk_base": chunk_base,
             "n_chunks": n_chunks, "n_slots": n_slots}
    return sched, (idx_stream, key_stream, w_stream)


def _pack_idx(idx_stream):
    """[NC, n_slots] -> [NC, 128, n_slots//16]: slot j at partition j%16,
    col j//16, replicated into all 8 groups of 16 partitions."""
    nc_, n_slots = idx_stream.shape
    cols = n_slots // 16
    wrapped = idx_stream.reshape(nc_, cols, 16).transpose(0, 2, 1)
    return np.ascontiguousarray(np.tile(wrapped, (1, 8, 1)))


# ---------------------------------------------------------------------------
# Numpy emulation of the device schedule (host-logic validation)
# ---------------------------------------------------------------------------

def emulate(x, edge_index, edge_time, node_time, edge_weight, W, b):
    sched, (idx_s, key_s, w_s) = _build_schedule(
        edge_index, edge_time, node_time, edge_weight)
    xf = np.asarray(x, dtype=np.float32)
    Wf = np.asarray(W, dtype=np.float32)
    bf = np.asarray(b, dtype=np.float32)
    nch = sched["nch"]
    chunk_base = sched["chunk_base"]
    out = np.zeros((T, N, D), dtype=np.float32)
    iota = np.arange(GR, dtype=np.float32)
    for k in range(NC):
        aggT = np.zeros((D, T * SLOTS_PER_T), dtype=np.float32)
        for sb in sched["sbs"]:
            t = sb["t"]
            for g in sb["groups"]:
                psum = np.zeros((D, GR), dtype=np.float32)
                for h in (0, 1):
                    for c in range(int(nch[t, h, g])):
                        ci = int(chunk_base[t, h, g]) + c
                        idx = idx_s[k, ci * CHUNK:(ci + 1) * CHUNK].astype(np.int64)
                        base = SPLIT if h else 0
                        msg = xf[t, base + idx, :]
                        key = key_s[k, ci]
                        w = w_s[k, ci]
                        sel = (key[:, None] == iota[None, :]) * w[:, None]
                        psum += msg.T @ sel
                sl = t * SLOTS_PER_T + g * GR
                aggT[:, sl:sl + GR] = psum
        for t in range(T):
            block = aggT[:, t * SLOTS_PER_T:(t + 1) * SLOTS_PER_T]
            outT = Wf.T @ block + bf[:, None]
            out[t, k * RANGE:(k + 1) * RANGE, :] = outT[:, :RANGE].T
    return out


# ---------------------------------------------------------------------------
# Bass kernel builder
# ---------------------------------------------------------------------------

def build_tile_kernel(tc, out_ap, ins, sched):
    """ins: dict with xt0..xt7 [N,128] bf16, idx [128, n_slots//16] i16,
    key/wgt [128, n_chunks] f32, iota [128, 64] bf16, wmat [64, 64] bf16,
    bias [64, 1] f32, ident [64, 64] f32.  out_ap: [T*RANGE, 64] f32."""
    from contextlib import ExitStack
    from concourse import bass, tile, mybir
    dt = mybir.dt
    nc = tc.nc
    nch = sched["nch"]
    chunk_base = sched["chunk_base"]

    with ExitStack() as ctx:
        const_p = ctx.enter_context(tc.tile_pool(name="const", bufs=1))
        msg_p = ctx.enter_context(tc.tile_pool(name="msg", bufs=2))
        aux_p = ctx.enter_context(tc.tile_pool(name="aux", bufs=2))
        sel_p = ctx.enter_context(tc.tile_pool(name="sel", bufs=4))
        agg_p = ctx.enter_context(tc.tile_pool(name="agg", bufs=1))
        stage_p = ctx.enter_context(tc.tile_pool(name="stage", bufs=3))
        psum_p = ctx.enter_context(tc.tile_pool(name="psum", bufs=4, space="PSUM"))
        psumt_p = ctx.enter_context(tc.tile_pool(name="psumt", bufs=2, space="PSUM"))

        iota_t = const_p.tile([128, GR], dt.bfloat16, tag="iota")
        nc.sync.dma_start(iota_t[:], ins["iota"][:])
        wmat_t = const_p.tile([D, D], dt.bfloat16, tag="wmat")
        nc.sync.dma_start(wmat_t[:], ins["wmat"][:])
        bias_t = const_p.tile([D, 1], dt.float32, tag="bias")
        nc.sync.dma_start(bias_t[:], ins["bias"][:])
        ident_t = const_p.tile([D, D], dt.float32, tag="ident")
        nc.sync.dma_start(ident_t[:], ins["ident"][:])

        aggT = agg_p.tile([D, T * SLOTS_PER_T], dt.bfloat16, tag="aggT")

        xt = [ins[f"xt{t}"] for t in range(T)]

        for sb in sched["sbs"]:
            t = sb["t"]
            lo0, lo1 = sb["lo"]
            hi0, hi1 = sb["hi"]
            nb = hi1 - lo0                     # total chunks in super-batch
            msg = msg_p.tile([128, SB_CHUNKS, 128], dt.bfloat16, tag="msg")
            # gather lo / hi
            for (c0, c1, base) in ((lo0, lo1, 0), (hi0, hi1, SPLIT)):
                nchk = c1 - c0
                if nchk == 0:
                    continue
                nidx = nchk * CHUNK
                idx_t = aux_p.tile([128, SB_CHUNKS * 8], dt.int16, tag="idx")
                nc.sync.dma_start(idx_t[:, :nidx // 16],
                                  ins["idx"][:, c0 * 8:c0 * 8 + nidx // 16])
                src_ap = xt[t][SPLIT:N, :] if base else xt[t][0:SPLIT, :]
                nc.gpsimd.dma_gather(
                    out_ap=msg[:, c0 - lo0:c0 - lo0 + nchk, :],
                    in_ap=src_ap,
                    idxs_ap=idx_t[:, :nidx // 16],
                    num_idxs=nidx,
                    num_idxs_reg=nidx,
                    elem_size=128,
                    single_packet=False,
                )
            key_t = aux_p.tile([128, SB_CHUNKS], dt.float32, tag="key")
            nc.sync.dma_start(key_t[:, :nb], ins["key"][:, lo0:lo0 + nb])
            w_t = aux_p.tile([128, SB_CHUNKS], dt.float32, tag="wgt")
            nc.sync.dma_start(w_t[:, :nb], ins["wgt"][:, lo0:lo0 + nb])

            for g in sb["groups"]:
                n_lo = int(nch[t, 0, g])
                n_hi = int(nch[t, 1, g])
                ntot = n_lo + n_hi
                psum = psum_p.tile([D, GR], dt.float32, tag="grp")
                done = 0
                for h, n_h in ((0, n_lo), (1, n_hi)):
                    cb = int(chunk_base[t, h, g])
                    for c in range(n_h):
                        ci = cb + c              # stream chunk id
                        pos = ci - lo0           # position in msg tile
                        sel = sel_p.tile([128, GR], dt.bfloat16, tag="sel")
                        nc.vector.tensor_scalar(
                            sel[:], iota_t[:],
                            key_t[:, ci - lo0:ci - lo0 + 1],
                            w_t[:, ci - lo0:ci - lo0 + 1],
                            mybir.AluOpType.is_equal, mybir.AluOpType.mult)
                        nc.tensor.matmul(
                            psum[:], msg[:, pos, 0:D], sel[:],
                            start=(done == 0), stop=(done == ntot - 1))
                        done += 1
                sl = t * SLOTS_PER_T + g * GR
                nc.scalar.activation(aggT[:, sl:sl + GR], psum[:],
                                     mybir.ActivationFunctionType.Copy)

        # Tail: per timestep @W, +bias, transpose, write out.
        for t in range(T):
            for s in range(0, SLOTS_PER_T, 512):
                w512 = min(512, SLOTS_PER_T - s)
                psw = psumt_p.tile([D, 512], dt.float32, tag="psw")
                nc.tensor.matmul(psw[:, :w512], wmat_t[:],
                                 aggT[:, t * SLOTS_PER_T + s:
                                      t * SLOTS_PER_T + s + w512],
                                 start=True, stop=True)
                outTs = stage_p.tile([D, 512], dt.float32, tag="outTs")
                nc.scalar.activation(outTs[:, :w512], psw[:, :w512],
                                     mybir.ActivationFunctionType.Identity,
                                     bias=bias_t[:])
                for s1 in range(0, w512, 128):
                    node0 = s + s1               # within this t's 6272 slots
                    if node0 >= RANGE:
                        break
                    nrow = min(128, RANGE - node0)
                    pst = psumt_p.tile([128, D], dt.float32, tag="pst")
                    nc.tensor.transpose(pst[:], outTs[:, s1:s1 + 128],
                                        ident_t[:])
                    st = stage_p.tile([128, D], dt.float32, tag="st")
                    nc.vector.tensor_copy(st[:], pst[:])
                    nc.sync.dma_start(
                        out_ap[t * RANGE + node0:t * RANGE + node0 + nrow, :],
                        st[:nrow, :])


# ---------------------------------------------------------------------------
# Top-level kernel
# ---------------------------------------------------------------------------

_CACHE = {}


def _declare_io(nc, dt, n_chunks, n_slots, null=False):
    in_aps = {}
    for t in range(T):
        in_aps[f"xt{t}"] = nc.dram_tensor(
            f"xt{t}", [N, 128], dt.bfloat16, kind="ExternalInput").ap()
    in_aps["idx"] = nc.dram_tensor(
        "idx", [128, n_slots // 16], dt.int16, kind="ExternalInput").ap()
    in_aps["key"] = nc.dram_tensor(
        "key", [128, n_chunks], dt.float32, kind="ExternalInput").ap()
    in_aps["wgt"] = nc.dram_tensor(
        "wgt", [128, n_chunks], dt.float32, kind="ExternalInput").ap()
    in_aps["iota"] = nc.dram_tensor(
        "iota", [128, GR], dt.bfloat16, kind="ExternalInput").ap()
    in_aps["wmat"] = nc.dram_tensor(
        "wmat", [D, D], dt.bfloat16, kind="ExternalInput").ap()
    in_aps["bias"] = nc.dram_tensor(
        "bias", [D, 1], dt.float32, kind="ExternalInput").ap()
    in_aps["ident"] = nc.dram_tensor(
        "ident", [D, D], dt.float32, kind="ExternalInput").ap()
    shape = [128, D] if null else [T * RANGE, D]
    out_ap = nc.dram_tensor("out", shape, dt.float32, kind="ExternalOutput").ap()
    return in_aps, out_ap


def _get_state(edge_index, edge_time, node_time, edge_weight):
    from concourse import bacc, tile, mybir
    dt = mybir.dt
    key = (edge_index.tobytes(), edge_time.tobytes(), node_time.tobytes(),
           edge_weight.tobytes())
    key = hash(key)
    if _CACHE.get("key") == key:
        return _CACHE["state"]

    sched, (idx_s, key_s, w_s) = _build_schedule(
        edge_index, edge_time, node_time, edge_weight)
    n_chunks, n_slots = sched["n_chunks"], sched["n_slots"]

    nc = bacc.Bacc("TRN2", target_bir_lowering=False, debug=False,
                   enable_asserts=False)
    in_aps, out_ap = _declare_io(nc, dt, n_chunks, n_slots)
    with tile.TileContext(nc) as tc:
        build_tile_kernel(tc, out_ap, in_aps, sched)
    if not nc.is_finalized():
        nc.finalize()

    # Null kernel: same inputs, trivial body (for transfer-overhead baseline).
    nc0 = bacc.Bacc("TRN2", target_bir_lowering=False, debug=False,
                    enable_asserts=False)
    in_aps0, out_ap0 = _declare_io(nc0, dt, n_chunks, n_slots, null=True)
    with tile.TileContext(nc0) as tc0:
        from contextlib import ExitStack
        with ExitStack() as c0:
            p0 = c0.enter_context(tc0.tile_pool(name="p0", bufs=1))
            t0_ = p0.tile([128, D], dt.float32, tag="t0")
            nc0.vector.memset(t0_[:], 0.0)
            nc0.sync.dma_start(t0_[0:D, :], in_aps0["ident"][:])
            nc0.sync.dma_start(out_ap0[:], t0_[:])
    if not nc0.is_finalized():
        nc0.finalize()

    state = {"sched": sched, "idx_s": idx_s, "key_s": key_s, "w_s": w_s,
             "nc": nc, "nc0": nc0,
             "idx_packed": _pack_idx(idx_s),
             "key_packed": key_s.transpose(0, 2, 1).copy(),
             "w_packed": w_s.transpose(0, 2, 1).copy()}
    _CACHE["key"] = key
    _CACHE["state"] = state
    return state


def _make_in_maps(state, x, W, b):
    import ml_dtypes
    bf16 = ml_dtypes.bfloat16
    xb = np.asarray(x).astype(bf16)
    xtab = np.concatenate([xb, xb], axis=2)               # [T, N, 128]
    iota_np = np.tile(np.arange(GR, dtype=np.float32)[None, :],
                      (128, 1)).astype(bf16)
    wmat_np = np.asarray(W).astype(bf16)
    bias_np = np.asarray(b).astype(np.float32).reshape(D, 1)
    ident_np = np.eye(D, dtype=np.float32)
    in_maps = []
    for k in range(NC):
        m = {f"xt{t}": xtab[t] for t in range(T)}
        m["idx"] = state["idx_packed"][k]
        m["key"] = state["key_packed"][k]
        m["wgt"] = state["w_packed"][k]
        m["iota"] = iota_np
        m["wmat"] = wmat_np
        m["bias"] = bias_np
        m["ident"] = ident_np
        in_maps.append(m)
    return in_maps


def kernel(x, edge_index, edge_time, node_time, edge_weight, W, b):
    from concourse.bass_utils import run_bass_kernel_spmd
    edge_index = np.asarray(edge_index)
    edge_time = np.asarray(edge_time)
    node_time = np.asarray(node_time)
    edge_weight = np.asarray(edge_weight)
    state = _get_state(edge_index, edge_time, node_time, edge_weight)
    in_maps = _make_in_maps(state, x, W, b)
    res = run_bass_kernel_spmd(state["nc"], in_maps, core_ids=list(range(NC)))
    out = np.zeros((T, N, D), dtype=np.float32)
    for k in range(NC):
        o = res.results[k]["out"].reshape(T, RANGE, D)
        out[:, k * RANGE:(k + 1) * RANGE, :] = o
    _CACHE["last_results"] = res
    return out


def null_run(x, edge_index, edge_time, node_time, edge_weight, W, b):
    """Same input transfer volume, trivial compute (timing baseline)."""
    from concourse.bass_utils import run_bass_kernel_spmd
    state = _get_state(np.asarray(edge_index), np.asarray(edge_time),
                       np.asarray(node_time), np.asarray(edge_weight))
    in_maps = _make_in_maps(state, x, W, b)
    res = run_bass_kernel_spmd(state["nc0"], in_maps, core_ids=list(range(NC)))
    return res.results[0]["out"]
